# revision 1
# baseline (speedup 1.0000x reference)
"""Trainium2 Bass kernel for nn_AdaptiveMultiGabor2DLayer.

Math (per channel c, with ic = indices[c]):
    W_l = (U_l @ V_l).reshape(NCHAN, IN_F, OUT_F)[ic]      (complex, l = 1,2)
    lin_l = x[c] @ W_l + b_l[ic]                           (complex [NPTS, OUT_F])
    out[c] = exp(1j*30*lin1 - 25*|lin1|^2 - 25*|lin2|^2)

Device formulation (all real):
    p  = 5*Re(lin1), q = 5*Im(lin1), p2 = 5*Re(lin2), q2 = 5*Im(lin2)
    Re(arg) = -6q - p^2 - q^2 - p2^2 - q2^2 = 9 - p^2 - (q+3)^2 - p2^2 - q2^2
    Im(arg) = 6p
    out = C * erf'(p) * erf'(q+3) * erf'(p2) * erf'(q2) * (cos(6p) + i sin(6p))
    where erf'(x) = (2/sqrt(pi)) e^{-x^2}  (ScalarE Derivative_Erf, one pass per
    quantity, per-partition bias folds the +bias terms), C = e^9 * (pi^2/16).

Layout: feature-major ("option A"). The matmul computes
    psum[j, n] = sum_k B[k', j] * xT[k', n]
with k' the re/im-interleaved input feature index (512 rows), j the output
feature (on partitions), n the points (free axis). B is host-prebuilt from
U/V per channel (pre-scaled by 5), so the device only runs the big matmuls
plus the pointwise epilogue. Sharding: 8 channels per NeuronCore (expert
parallel, no collectives).
"""

import math
import sys

import numpy as np

NCORES = 8
NCHAN = 64
NPTS = 2048
IN_F = 256
OUT_F = 256
CH_PER_CORE = NCHAN // NCORES


def _ensure_path():
    try:
        import concourse  # noqa: F401
    except ImportError:
        for p in ("/opt/trn_rl_repo", "/root/.axon_site/_ro/trn_rl_repo"):
            if p not in sys.path:
                sys.path.insert(0, p)


_NC_CACHE = {}


def build_nc(nch=CH_PER_CORE, npts=NPTS, ch_batch=1):
    """Build the per-core Bass graph (SPMD: same graph on all 8 cores)."""
    key = (nch, npts, ch_batch)
    if key in _NC_CACHE:
        return _NC_CACHE[key]
    _ensure_path()
    import concourse.bacc as bacc
    import concourse.tile as tile
    from concourse import mybir

    dt = mybir.dt
    AF = mybir.ActivationFunctionType
    OP = mybir.AluOpType

    NW = min(1024, npts)          # width of one epilogue unit (<= 2 psum banks)
    n_nt = npts // NW
    nsplit = max(1, NW // 512)    # matmul N chunks per unit
    mmw = min(512, NW)
    CBIG = math.exp(9.0) * (math.sqrt(math.pi) / 2.0) ** 4
    ch_batch = min(ch_batch, nch)

    nc = bacc.Bacc("TRN2", target_bir_lowering=False)
    xt_d = nc.declare_dram_parameter("xt", [nch, 512, npts], dt.bfloat16, isOutput=False)
    bm_d = nc.declare_dram_parameter("bmat", [nch, 2, 512, 512], dt.bfloat16, isOutput=False)
    bv_d = nc.declare_dram_parameter("biasv", [nch, 7, 256], dt.float32, isOutput=False)
    out_d = nc.declare_dram_parameter("out", [nch, 2, 256, npts], dt.bfloat16, isOutput=True)

    def bcol(ch, v, jh):
        return (ch * 7 + v) * 2 + jh

    with tile.TileContext(nc) as tc:
        with (
            tc.tile_pool(name="xpool", bufs=2) as xpool,
            tc.tile_pool(name="bpool", bufs=2) as bpool,
            tc.tile_pool(name="cpool", bufs=1) as cpool,
            tc.tile_pool(name="spool", bufs=2) as spool,
            tc.tile_pool(name="stpool", bufs=1) as stpool,
            tc.tile_pool(name="pspool", bufs=max(2, 8 // max(1, min(1024, npts) // 512)), space="PSUM") as pspool,
        ):
            biast = cpool.tile([128, nch * 7 * 2], dt.float32)
            nc.sync.dma_start(
                out=biast[:], in_=bv_d[:].rearrange("c v (h p) -> p (c v h)", p=128)
            )

            def chain(inst):
                # Serialize ScalarE ops in program order so activation
                # table-set loads only happen at our phase boundaries.
                tc.chain_iter_dep("act_order", inst.ins if hasattr(inst, "ins") else inst)

            n_stage = ch_batch * 2 * 2 + 1

            PSW = min(1024, npts)   # psum quantity tiles (2 banks, 4 slots)
            n_pw = npts // PSW
            DW = min(1024, npts)    # DVE op width (2x mode holds at <=1024)
            n_dw = npts // DW

            def emit_phase_T(staged):
                for ch, jh, p6, mh in staged:
                    # s = sin(3p+b), c = cos(3p+b); E = C*mh
                    # out_i = E*sin(6p) = (2C*mh*s)*c ; out_r = E*cos(6p) = C*mh - (2C*mh*s)*s
                    sn = spool.tile([128, npts], dt.bfloat16, tag="sn", name=f"sn_{ch}_{jh}")
                    inst = nc.scalar.activation(
                        sn[:], p6[:], AF.Sin,
                        bias=biast[:, bcol(ch, 5, jh) : bcol(ch, 5, jh) + 1],
                    )
                    chain(inst)
                    cs = spool.tile([128, npts], dt.bfloat16, tag="cs", name=f"cs_{ch}_{jh}")
                    inst = nc.scalar.activation(
                        cs[:], p6[:], AF.Sin,
                        bias=biast[:, bcol(ch, 6, jh) : bcol(ch, 6, jh) + 1],
                    )
                    chain(inst)
                    for dw in range(n_dw):
                        dsl = slice(dw * DW, dw * DW + DW)
                        # u = mh*s ; uC = 2C*u ; out_i = uC*c ; v = uC*s (=2C*mh*s^2)
                        # mhC = C*mh ; out_r = mhC - v = C*mh*(1-2s^2)
                        u = spool.tile([128, DW], dt.bfloat16, tag="u", name=f"u_{ch}_{jh}_{dw}")
                        nc.vector.tensor_tensor(out=u[:], in0=mh[:, dsl], in1=sn[:, dsl], op=OP.mult)
                        uC = spool.tile([128, DW], dt.bfloat16, tag="uC", name=f"uC_{ch}_{jh}_{dw}")
                        nc.vector.tensor_scalar(
                            out=uC[:], in0=u[:], scalar1=2.0 * CBIG, scalar2=None, op0=OP.mult
                        )
                        oii = spool.tile([128, DW], dt.bfloat16, tag="oii", name=f"oii_{ch}_{jh}_{dw}")
                        nc.vector.tensor_tensor(out=oii[:], in0=uC[:], in1=cs[:, dsl], op=OP.mult)
                        v = spool.tile([128, DW], dt.bfloat16, tag="v", name=f"v_{ch}_{jh}_{dw}")
                        nc.vector.tensor_tensor(out=v[:], in0=uC[:], in1=sn[:, dsl], op=OP.mult)
                        mhC = spool.tile([128, DW], dt.bfloat16, tag="mhC", name=f"mhC_{ch}_{jh}_{dw}")
                        nc.vector.tensor_scalar(
                            out=mhC[:], in0=mh[:, dsl], scalar1=CBIG, scalar2=None, op0=OP.mult
                        )
                        orr = spool.tile([128, DW], dt.bfloat16, tag="orr", name=f"orr_{ch}_{jh}_{dw}")
                        nc.vector.tensor_tensor(out=orr[:], in0=mhC[:], in1=v[:], op=OP.subtract)
                        nc.sync.dma_start(
                            out=out_d[ch, 0, jh * 128 : jh * 128 + 128, dsl], in_=orr[:]
                        )
                        nc.sync.dma_start(
                            out=out_d[ch, 1, jh * 128 : jh * 128 + 128, dsl], in_=oii[:]
                        )

            _gdma = {}

            def emit_phase_G(ch, staged, only_jh=None):
                if ch not in _gdma:
                    # DMA order matters for the pipeline head: the first matmul
                    # needs only xt chunk 0 + the l=0 weight half (1 MB), so issue
                    # those first and stream the rest behind them.
                    xt_ks = [
                        xpool.tile([128, npts], dt.bfloat16, tag=f"xt{kc}", name=f"xt{kc}_{ch}")
                        for kc in range(4)
                    ]
                    bm_ls = [
                        bpool.tile([128, 4, 512], dt.bfloat16, tag=f"bm{l}", name=f"bm{l}_{ch}")
                        for l in range(2)
                    ]
                    nc.sync.dma_start(out=xt_ks[0][:], in_=xt_d[ch, 0:128, :])
                    nc.sync.dma_start(
                        out=bm_ls[0][:], in_=bm_d[ch, 0].rearrange("(k p) j -> p k j", p=128)
                    )
                    for kc in range(1, 4):
                        nc.sync.dma_start(
                            out=xt_ks[kc][:], in_=xt_d[ch, kc * 128 : kc * 128 + 128, :]
                        )
                    nc.sync.dma_start(
                        out=bm_ls[1][:], in_=bm_d[ch, 1].rearrange("(k p) j -> p k j", p=128)
                    )
                    _gdma[ch] = (xt_ks, bm_ls)
                xt_ks, bm_ls = _gdma[ch]
                for jh in ([only_jh] if only_jh is not None else range(2)):
                    p6 = stpool.tile([128, npts], dt.float16, tag="p6", bufs=n_stage)
                    mh = stpool.tile([128, npts], dt.bfloat16, tag="mh", bufs=n_stage)
                    qdefs = [
                        (0, jh * 128),        # P:  5*Re(lin1)
                        (0, 256 + jh * 128),  # Q:  5*Im(lin1)
                        (1, jh * 128),        # R:  5*Re(lin2)
                        (1, 256 + jh * 128),  # S:  5*Im(lin2)
                    ]
                    g = [
                        spool.tile([128, npts], dt.bfloat16, tag=f"g{qi}", name=f"g{qi}_{ch}_{jh}")
                        for qi in range(4)
                    ]
                    for pw in range(n_pw):
                        ps = [
                            pspool.tile(
                                [128, PSW], dt.float32, tag="q", name=f"ps{qi}_{ch}_{jh}_{pw}"
                            )
                            for qi in range(4)
                        ]
                        for qi, (l, cb) in enumerate(qdefs):
                            for ns in range(PSW // mmw):
                                for kc in range(4):
                                    nc.tensor.matmul(
                                        ps[qi][:, ns * mmw : ns * mmw + mmw],
                                        bm_ls[l][:, kc, cb : cb + 128],
                                        xt_ks[kc][:, pw * PSW + ns * mmw : pw * PSW + (ns + 1) * mmw],
                                        start=(kc == 0),
                                        stop=(kc == 3),
                                    )
                            # consume this quantity: Gaussian (ACT) right away
                            psl = slice(pw * PSW, pw * PSW + PSW)
                            inst = nc.scalar.activation(
                                g[qi][:, psl],
                                ps[qi][:],
                                AF.Derivative_Erf,
                                bias=biast[:, bcol(ch, qi, jh) : bcol(ch, qi, jh) + 1],
                                scale=1.0,
                            )
                            chain(inst)
                            if qi == 0:
                                nc.vector.tensor_scalar(
                                    out=p6[:, psl],
                                    in0=ps[0][:],
                                    scalar1=3.0,
                                    scalar2=biast[:, bcol(ch, 4, jh) : bcol(ch, 4, jh) + 1],
                                    op0=OP.mult,
                                    op1=OP.add,
                                )
                    # combines at DVE-friendly width
                    for dw in range(n_dw):
                        dsl = slice(dw * DW, dw * DW + DW)
                        m1 = spool.tile([128, DW], dt.bfloat16, tag="m1", name=f"m1_{ch}_{jh}_{dw}")
                        nc.vector.tensor_tensor(out=m1[:], in0=g[0][:, dsl], in1=g[1][:, dsl], op=OP.mult)
                        m2 = spool.tile([128, DW], dt.bfloat16, tag="m2", name=f"m2_{ch}_{jh}_{dw}")
                        nc.vector.tensor_tensor(out=m2[:], in0=g[2][:, dsl], in1=g[3][:, dsl], op=OP.mult)
                        nc.vector.tensor_tensor(out=mh[:, dsl], in0=m1[:], in1=m2[:], op=OP.mult)
                    staged.append((ch, jh, p6, mh))


            pending = None
            for b0 in range(0, nch, ch_batch):
                chs = list(range(b0, min(b0 + ch_batch, nch)))
                staged = []
                # ---- phase G: matmuls + Gaussian factors (erf_derivative set)
                # The previous batch's trig block is inserted between the two
                # jh-units so the PE has a fresh PSUM runway on both sides of
                # the trig table phase (same table-load count).
                for i, ch in enumerate(chs):
                    emit_phase_G(ch, staged, only_jh=0)
                    if i == 0 and pending:
                        emit_phase_T(pending)
                        pending = None
                    emit_phase_G(ch, staged, only_jh=1)
                if pending:
                    emit_phase_T(pending)
                pending = staged
            emit_phase_T(pending)

    nc.finalize()
    _NC_CACHE[key] = nc
    return nc


def prepare_inputs(x, indices, U1, V1, b1, U2, V2, b2):
    """Host-side marshaling: gather per-channel params, build scaled weight
    matrices, transpose x to feature-major. Returns (xt, bmat, biasv)."""
    import ml_dtypes

    bf16 = ml_dtypes.bfloat16
    x = np.asarray(x)
    indices = np.asarray(indices).astype(np.int64)
    U1 = np.asarray(U1); V1 = np.asarray(V1); b1 = np.asarray(b1)
    U2 = np.asarray(U2); V2 = np.asarray(V2); b2 = np.asarray(b2)
    nch, npts, inf = x.shape

    B = np.empty((nch, 2, 2 * inf, 2 * OUT_F), np.float32)
    for li, (U, V) in enumerate(((U1, V1), (U2, V2))):
        W = (U[indices] @ V).reshape(nch, inf, OUT_F)
        Wr = 5.0 * np.ascontiguousarray(W.real)
        Wi = 5.0 * np.ascontiguousarray(W.imag)
        B[:, li, 0::2, :OUT_F] = Wr
        B[:, li, 1::2, :OUT_F] = -Wi
        B[:, li, 0::2, OUT_F:] = Wi
        B[:, li, 1::2, OUT_F:] = Wr
    bmat = B.astype(bf16)

    bg1 = b1[indices, 0, :]
    bg2 = b2[indices, 0, :]
    biasv = np.stack(
        [
            5.0 * bg1.real,          # bias for P gaussian
            5.0 * bg1.imag + 3.0,    # bias for Q gaussian (completed square)
            5.0 * bg2.real,          # R
            5.0 * bg2.imag,          # S
            15.0 * bg1.real,         # half-phase offset: p3 = 3*P_raw + 15*Re(b1)
            np.zeros_like(bg1.real),            # sin bias (0)
            np.full_like(bg1.real, math.pi / 2),  # cos bias (pi/2)
        ],
        axis=1,
    ).astype(np.float32)

    xv = x.view(np.float32).reshape(nch, npts, 2 * inf)
    xt = np.ascontiguousarray(xv.transpose(0, 2, 1)).astype(bf16)
    return xt, bmat, biasv


def combine_output(full, npts=NPTS):
    """full: [nch, 2, 256, npts] bf16/f32 -> complex64 [nch, npts, 256]."""
    fr = np.asarray(full).astype(np.float32)
    out = (fr[:, 0] + 1j * fr[:, 1]).astype(np.complex64)
    return np.ascontiguousarray(out.transpose(0, 2, 1))


def kernel(x, indices, U1, V1, b1, U2, V2, b2):
    _ensure_path()
    from concourse.bass_utils import run_bass_kernel_spmd

    xt, bmat, biasv = prepare_inputs(x, indices, U1, V1, b1, U2, V2, b2)
    nc = build_nc()
    in_maps = []
    for c in range(NCORES):
        sl = slice(c * CH_PER_CORE, (c + 1) * CH_PER_CORE)
        in_maps.append(
            {
                "xt": np.ascontiguousarray(xt[sl]),
                "bmat": np.ascontiguousarray(bmat[sl]),
                "biasv": np.ascontiguousarray(biasv[sl]),
            }
        )
    res = run_bass_kernel_spmd(nc, in_maps, list(range(NCORES)))
    outs = [np.asarray(res.results[i]["out"]) for i in range(NCORES)]
    full = np.concatenate(outs, axis=0)
    return combine_output(full)

